# revision 20
# baseline (speedup 1.0000x reference)
"""Distributed Trainium2 Bass kernel for nn_Attention_79766132621772.

Reference computation (all fp32):
    B, L, D, H, HD = 2, 2048, 2048, 16, 128
    qkv = (x @ w_qkv).reshape(B, L, 3, H, HD)
    q, k = rope(q), rope(k)                       # positions along L
    att = softmax(q @ k^T / sqrt(HD))             # per (b, h)
    out = (att @ v).reshape(B, L, D) @ w_proj

Sharding: tensor-parallel over heads for QKV+attention (16 heads / 8 cores =
2 heads per core), then an AllToAll redistributes the per-head attention
outputs into (token-group, output-column) shards for the projection: core r
projects token group r//2 (512 tokens per batch) against output columns
[1024*(r%2), 1024*(r%2+1)) with a resident half-w_proj. The AllToAll moves
only each core's 2-head oT slices (3.7 MB/core wire) instead of the 14.7
MB/core a ReduceScatter of full-D projection partials would need, and the
projection PE cost is unchanged. The host concatenates the 8 disjoint
(token, column) output shards.

Precision: bf16 operands everywhere upstream of the fp32 PSUM accumulators
(x, w_qkv, q/k/v, P, w_proj). Emulated end-to-end rel err 6e-3, and bf16
halves the x DMA stream and SBUF footprint vs fp32 while running at the same
1 cycle/row TensorE rate as fp32r.

Schedule: the PE is the bottleneck (roofline ~786k cycles/core). Stage 2 is
gated by ScalarE exp, so the emission order software-pipelines the engines:
the next batch's QKV matmuls and the previous batch's projection are
drip-fed as filler into the exp-gated stage-2 units, keeping the PE queue
fed while ScalarE chews exp. Nothing in the steady-state path touches
GpSimd (the collective occupies its in-order queue for the whole transfer,
so anything queued behind it would stall): row sums accumulate on DVE, and
the partition reduce uses a ones-MATRIX matmul whose output already
broadcasts the rowsum to all 128 partitions, so normalization is just a
DVE reciprocal + multiply. (Tried and reverted: fp8 DoubleRow — attention
averaging shrinks signal and noise equally, ~2.5-8%% output error; per-head
AllToAll splits — two extra collective launches cost more than the overlap
they buy; rot-matmul elimination via partition-offset swap copies — walrus
rejects the IR and the swap work gates whichever engine hosts it.)
"""

import os
import ml_dtypes
import numpy as np

import concourse.bass as bass
import concourse.tile as tile
from concourse import bacc, mybir
from concourse._compat import axon_active
from concourse.bass_utils import run_bass_kernel_spmd

B, L, D, H = 2, 2048, 2048, 16
HD = 128
NCORES = 8
HPC = H // NCORES          # heads per core = 2
T = B * L                  # total tokens = 4096
F32 = mybir.dt.float32
F32R = mybir.dt.float32r
BF16 = mybir.dt.bfloat16
SCALE = 1.0 / float(np.sqrt(HD))

_CHUNK = 512               # q/token chunk width (moving dim of matmuls)
_NKT = D // 128            # 16 contraction tiles for D=2048
_NCH = L // _CHUNK         # 4 chunks per batch
NTT = L // 128             # token tiles per batch = 16
TOKG = 512                 # tokens per group per batch (4 groups, 2 cores each)
COLW = D // 2              # output column shard width = 1024


def _build(reps=1, collective=True):
    # Native (non-axon) execution needs debug=True for the BassDebugger; the
    # axon/PJRT client path cannot host one and needs debug=False.
    nc = bacc.Bacc(
        "TRN2",
        target_bir_lowering=False,
        debug=not axon_active(),
        enable_asserts=False,
        num_devices=NCORES,
    )

    # ---- kernel I/O (per core) ----
    xT_d = nc.declare_dram_parameter("xT", [B, D, L], BF16, isOutput=False)
    wqkv_d = nc.declare_dram_parameter("w_qkv", [D, 6 * HD], BF16, isOutput=False)
    wproj_d = nc.declare_dram_parameter("w_proj", [D, COLW], BF16, isOutput=False)
    cos_d = nc.declare_dram_parameter("cos", [HD, L], BF16, isOutput=False)
    sin_d = nc.declare_dram_parameter("sin", [HD, L], BF16, isOutput=False)
    rot_d = nc.declare_dram_parameter("rot", [HD, HD], F32, isOutput=False)
    out_d = nc.declare_dram_parameter("out", [B * TOKG, COLW], F32, isOutput=True)

    with tile.TileContext(nc) as tc:
        _emit(nc, tc, xT_d, wqkv_d, wproj_d, cos_d, sin_d, rot_d, out_d, reps, collective)

    nc.compile()
    return nc


def _emit(nc, tc, xT_d, wqkv_d, wproj_d, cos_d, sin_d, rot_d, out_d, reps=1, collective=True):
    fdma = nc.sync.dma_start

    singles = tc.alloc_tile_pool(name="singles", bufs=1)
    # w_qkv in [128, kt, col] bf16 layout; cols: q_h0 q_h1 k_h0 k_h1 v_h0 v_h1
    w_sb = singles.tile([128, _NKT, 6 * HD], BF16)
    _wq_r = wqkv_d.ap().rearrange("(t p) c -> p t c", p=128)
    # startup-critical: only the first qk accumulation's own 128-column slice
    # of every k-tile; everything else is deferred until after the first x
    # chunk's DMAs are queued (the SP queue is in-order, so early non-critical
    # loads would delay the PE's first matmul)
    for _g in range(4):
        fdma(
            out=w_sb[:, 4 * _g : 4 * _g + 4, 0:128],
            in_=_wq_r[:, 4 * _g : 4 * _g + 4, 0:128],
        )
    wproj_sb = singles.tile([128, _NKT, COLW], BF16)
    cos_sb = singles.tile([HD, L], BF16)
    sin_sb = singles.tile([HD, L], BF16)
    rot_sb = singles.tile([HD, HD], F32R)
    ones_f32 = singles.tile([128, 128], F32)
    nc.vector.memset(ones_f32, 1.0)
    ones_sb = singles.tile([128, 128], F32R)
    nc.vector.tensor_copy(out=ones_sb, in_=ones_f32)

    def deferred_singles():
        fdma(out=rot_sb, in_=rot_d.ap().bitcast(F32R))
        for _cb in range(1, 6):
            fdma(
                out=w_sb[:, :, _cb * 128 : (_cb + 1) * 128],
                in_=_wq_r[:, :, _cb * 128 : (_cb + 1) * 128],
            )
        fdma(out=cos_sb, in_=cos_d.ap())
        fdma(out=sin_sb, in_=sin_d.ap())
        # w_proj is only needed at stage 3; emit its load last
        fdma(out=wproj_sb, in_=wproj_d.ap().rearrange("(t p) c -> p t c", p=128))

    # DRAM AllToAll buffers, one pair per batch: 8 blocks of [2*HD, TOKG];
    # block j carries this core's 2 heads x token group j//2 (destination j
    # projects those tokens against its column shard)
    dram = tc.alloc_tile_pool(name="dram", bufs=1, space="DRAM")
    a2a_in = dram.tile(
        [NCORES * HPC * HD, B * TOKG], BF16, tag="ain", name="a2a_in"
    )
    a2a_out = dram.tile(
        [NCORES * HPC * HD, B * TOKG], BF16, tag="aout", name="a2a_out"
    )

    # qT/kT/v double-buffered so batch b+1's stage 1 overlaps batch b's
    # attention
    per_b = tc.alloc_tile_pool(name="per_b", bufs=2)
    xp = tc.alloc_tile_pool(name="xp", bufs=int(os.environ.get("XG_BUFS", "19")))
    qs = tc.alloc_tile_pool(name="qs", bufs=int(os.environ.get("QS_BUFS", "2")))
    rp = tc.alloc_tile_pool(name="rp", bufs=int(os.environ.get("RP_BUFS", "2")))
    pp = tc.alloc_tile_pool(name="pp", bufs=int(os.environ.get("PP_BUFS", "4")))
    ap_ = tc.alloc_tile_pool(name="ap", bufs=2)
    op = tc.alloc_tile_pool(name="op", bufs=int(os.environ.get("OT_BUFS", "3")))
    at = tc.alloc_tile_pool(name="at", bufs=int(os.environ.get("AT_BUFS", "2")))
    ps_s = tc.alloc_tile_pool(name="ps_s", bufs=3, space="PSUM")
    ps_o = tc.alloc_tile_pool(name="ps_o", bufs=2, space="PSUM")
    ps_qkv = tc.alloc_tile_pool(name="ps_qkv", bufs=1, space="PSUM")
    ps_r = tc.alloc_tile_pool(name="ps_r", bufs=1, space="PSUM")

    state = {}  # (rep, b) -> dict of per-batch tiles

    def gen_s1(rep, b):
        """Stage 1 QKV+RoPE emitter; yields every ~2 PE matmuls so it can be
        drip-fed as filler into the exp-gated stage-2 units."""
        qT_sb = per_b.tile([128, HPC, L], BF16, tag="qT", name=f"qT_{rep}_{b}")
        kT_sb = per_b.tile([128, HPC, L], BF16, tag="kT", name=f"kT_{rep}_{b}")
        # v in [tok%128, tok_tile, head, HD] bf16 layout
        v_sb = per_b.tile([128, NTT, HPC, HD], BF16, tag="v", name=f"v_{rep}_{b}")
        state[(rep, b)] = dict(qT=qT_sb, kT=kT_sb, v=v_sb)

        xT_b = xT_d.ap()[b].rearrange("(t p) l -> p t l", p=128)  # [128,16,L]
        for ch in range(_NCH):
            c0 = ch * _CHUNK
            xg = []
            for g in range(_NKT):
                xgt = xp.tile([128, _CHUNK], BF16, tag=f"xg{b}", name=f"xg_{rep}_{b}_{ch}_{g}")
                fdma(out=xgt, in_=xT_b[:, g, c0 : c0 + _CHUNK])
                xg.append(xgt)

            # q/k in transposed [dim, token] layout, RoPE on eviction
            for ct in range(2 * HPC):
                dst = qT_sb if ct < HPC else kT_sb
                h = ct % HPC
                pq = ps_qkv.tile([128, _CHUNK], F32, tag=f"pqk{b}", name=f"pqk_{rep}_{b}_{ch}_{ct}")
                for kt in range(_NKT):
                    nc.tensor.matmul(
                        out=pq,
                        lhsT=w_sb[:, kt, ct * 128 : ct * 128 + 128],
                        rhs=xg[kt],
                        start=(kt == 0),
                        stop=(kt == _NKT - 1),
                    )
                    if kt % 2 == 1:
                        yield
                # evict, then rot = P_rot @ q via constant matmul
                qsb = qs.tile([128, _CHUNK], F32R, tag="qsb", name=f"qsb_{rep}_{b}_{ch}_{ct}")
                nc.scalar.copy(out=qsb, in_=pq)
                # shares ps_r's single bank with the rowsum-broadcast pr
                # tiles; both have atomic alloc-to-last-reader lifetimes, so
                # the slot rotation only ever waits on already-emitted work
                prot = ps_r.tile(
                    [128, _CHUNK], F32, tag="pr", name=f"prot_{rep}_{b}_{ch}_{ct}"
                )
                nc.tensor.matmul(out=prot, lhsT=rot_sb, rhs=qsb, start=True, stop=True)
                # q' = q*cos + rot*sin (single bf16 rounding at the end)
                cosc = cos_sb[:, c0 : c0 + _CHUNK]
                sinc = sin_sb[:, c0 : c0 + _CHUNK]
                dstc = dst[:, h, c0 : c0 + _CHUNK]
                t1 = rp.tile([128, _CHUNK], F32, tag="rt", name=f"rt_{rep}_{b}_{ch}_{ct}")
                t2 = rp.tile([128, _CHUNK], F32, tag="rt2", name=f"rt2_{rep}_{b}_{ch}_{ct}")
                nc.vector.tensor_mul(out=t1, in0=prot, in1=sinc)
                nc.vector.tensor_mul(out=t2, in0=qsb.bitcast(F32), in1=cosc)
                nc.vector.tensor_add(out=dstc, in0=t2, in1=t1)
                yield

            # v in [token, col] bf16 layout
            for tt in range(_CHUNK // 128):
                pv = ps_qkv.tile([128, HPC * HD], F32, tag=f"pqk{b}", name=f"pv_{rep}_{b}_{ch}_{tt}")
                for kt in range(_NKT):
                    nc.tensor.matmul(
                        out=pv,
                        lhsT=xg[kt][:, tt * 128 : tt * 128 + 128],
                        rhs=w_sb[:, kt, 2 * HPC * 128 :],
                        start=(kt == 0),
                        stop=(kt == _NKT - 1),
                    )
                    if kt % 4 == 3:
                        yield
                gt = ch * (_CHUNK // 128) + tt
                nc.scalar.copy(
                    out=v_sb[:, gt, :, :].rearrange("p h d -> p (h d)"), in_=pv
                )
                yield

    fil = []  # deque of filler generators

    def take(n):
        while n > 0 and fil:
            try:
                next(fil[0])
                n -= 1
            except StopIteration:
                fil.pop(0)

    def emit_s2_unit(rep, b, h, qc):
        st = state[(rep, b)]
        q0 = qc * _CHUNK
        qT_c = st["qT"][:, h, q0 : q0 + _CHUNK]
        po = ps_o.tile([128, _CHUNK], F32, tag="po", name=f"po_{rep}_{b}_{h}_{qc}")
        acc = ap_.tile([128, _CHUNK], F32R, tag="acc", name=f"acc_{rep}_{b}_{h}_{qc}")
        for kt in range(_NKT):
            psS = ps_s.tile([128, _CHUNK], F32, tag="ps", name=f"ps_{rep}_{b}_{h}_{qc}_{kt}")
            nc.tensor.matmul(
                out=psS,
                lhsT=st["kT"][:, h, kt * 128 : kt * 128 + 128],
                rhs=qT_c,
                start=True,
                stop=True,
            )
            pt = pp.tile([128, _CHUNK], BF16, tag="pt", name=f"pt_{rep}_{b}_{h}_{qc}_{kt}")
            nc.scalar.activation(
                out=pt, in_=psS, func=mybir.ActivationFunctionType.Exp,
                scale=SCALE,
            )
            nc.tensor.matmul(
                out=po,
                lhsT=st["v"][:, kt, h, :],
                rhs=pt,
                start=(kt == 0),
                stop=(kt == _NKT - 1),
            )
            # row-sum accumulation on DVE (NOT GpSimd: the collective
            # occupies the in-order GpSimd queue for its whole duration)
            if kt == 0:
                nc.vector.tensor_copy(out=acc, in_=pt)
            else:
                nc.vector.tensor_add(out=acc, in0=acc.bitcast(F32), in1=pt)
            take(1)
        # partition-reduce the column sums with a ones-MATRIX matmul: every
        # output partition gets the same rowsum, so no GpSimd broadcast is
        # needed (GpSimd stays free for the AllToAll collectives)
        pr = ps_r.tile([128, _CHUNK], F32, tag="pr", name=f"pr_{rep}_{b}_{h}_{qc}")
        nc.tensor.matmul(out=pr, lhsT=ones_sb, rhs=acc, start=True, stop=True)
        rbc = rp.tile([128, _CHUNK], F32, tag="rbc", name=f"rbc_{rep}_{b}_{h}_{qc}")
        nc.vector.reciprocal(out=rbc, in_=pr)
        ot = op.tile([128, _CHUNK], BF16, tag="ot", name=f"ot_{rep}_{b}_{h}_{qc}")
        nc.vector.tensor_mul(out=ot, in0=po, in1=rbc)
        # ship this unit's oT slice to both column-shard owners of token
        # group qc: AllToAll block rows [256*j + 128*h, +128) for j = 2qc,
        # 2qc+1
        for j in (2 * qc, 2 * qc + 1):
            r0 = (HPC * HD) * j + HD * h
            fdma(
                out=a2a_in[r0 : r0 + HD, b * TOKG : (b + 1) * TOKG], in_=ot
            )

    def emit_a2a(rep):
        if collective:
            nc.gpsimd.collective_compute(
                "AllToAll",
                mybir.AluOpType.bypass,
                replica_groups=[list(range(NCORES))],
                ins=[a2a_in.opt()],
                outs=[a2a_out.opt()],
            )

    def gen_proj(rep, b):
        """Project this core's token group of batch b (512 tokens x COLW
        columns, full-D contraction). Each quantum is atomic per token tile
        (no pool tile stays live across a yield, so interleaved emission
        from other generators can never create a forward slot-wait)."""
        src = a2a_out if collective else a2a_in
        src_r = src.rearrange("(t p) c -> p t c", p=128)  # [128,16,B*TOKG]
        for tt in range(TOKG // 128):
            a2a_t = at.tile([128, _NKT, 128], BF16, tag="at", name=f"at_{rep}_{b}_{tt}")
            c0 = b * TOKG + tt * 128
            fdma(out=a2a_t, in_=src_r[:, :, c0 : c0 + 128])
            for nch in range(COLW // _CHUNK):
                pout = ps_s.tile([128, _CHUNK], F32, tag="ps", name=f"pout_{rep}_{b}_{tt}_{nch}")
                for kt in range(_NKT):
                    nc.tensor.matmul(
                        out=pout,
                        lhsT=a2a_t[:, kt, :],
                        rhs=wproj_sb[:, kt, nch * _CHUNK : (nch + 1) * _CHUNK],
                        start=(kt == 0),
                        stop=(kt == _NKT - 1),
                    )
                fout = qs.tile([128, _CHUNK], F32, tag="fout", name=f"fo_{rep}_{b}_{tt}_{nch}")
                # alternate eviction between ScalarE and VectorE
                if nch % 2 == 0:
                    nc.scalar.copy(out=fout, in_=pout)
                else:
                    nc.vector.tensor_copy(out=fout, in_=pout)
                fdma(
                    out=out_d.ap()[
                        b * TOKG + tt * 128 : b * TOKG + (tt + 1) * 128,
                        nch * _CHUNK : (nch + 1) * _CHUNK,
                    ],
                    in_=fout,
                )
            yield

    gens1 = {}

    def s1g(rep, b):
        if (rep, b) not in gens1:
            gens1[(rep, b)] = gen_s1(rep, b)
        return gens1[(rep, b)]

    for rep in range(reps):
        if rep == 0:
            next(s1g(0, 0))      # queue the first x chunk's DMAs first
            deferred_singles()   # then the non-critical parameter loads
        for _ in s1g(rep, 0):    # finish stage 1 of b0, draining queued proj
            take(1)
        for b in range(B):
            # queue the next batch's stage 1 as filler for this batch's
            # exp-gated attention units
            nrep, nb = (rep, 1) if b == 0 else (rep + 1, 0)
            if nrep < reps:
                fil.append(s1g(nrep, nb))
            for qc in range(_NCH):
                for h in range(HPC):
                    emit_s2_unit(rep, b, h, qc)
            if b == 0:
                for _ in s1g(rep, 1):
                    take(1)
            else:
                emit_a2a(rep)
                # prioritize the projections over the next stage 1
                fil.insert(0, gen_proj(rep, 0))
                fil.insert(1, gen_proj(rep, 1))
        # drain any remaining filler at the rep boundary? No — leave it for
        # the next rep's stage-2 gaps; force-drain only at the very end.
    while fil:
        take(100)

    for p in (ps_r, ps_qkv, ps_o, ps_s, at, op, ap_, pp, rp, qs, xp, per_b, dram, singles):
        p.release()


def _make_inputs(x, w_qkv, w_proj):
    x = np.asarray(x, dtype=np.float32)
    w_qkv = np.asarray(w_qkv, dtype=np.float32)
    w_proj = np.asarray(w_proj, dtype=np.float32)
    xT = np.ascontiguousarray(x.transpose(0, 2, 1)).astype(ml_dtypes.bfloat16)

    freqs = (1.0 / (10000.0 ** (np.arange(0, HD, 2, dtype=np.float32) / HD))).astype(
        np.float32
    )
    f = np.outer(np.arange(L, dtype=np.float32), freqs).astype(np.float32)  # [L, 64]
    cos_t = np.ascontiguousarray(np.repeat(np.cos(f), 2, axis=1).T.astype(ml_dtypes.bfloat16))
    sin_t = np.ascontiguousarray(np.repeat(np.sin(f), 2, axis=1).T.astype(ml_dtypes.bfloat16))

    # rot param R = P_rot^T, where rot(q) = P_rot @ q swaps pairs:
    # rot[2i] = -q[2i+1], rot[2i+1] = q[2i]
    R = np.zeros((HD, HD), dtype=np.float32)
    for i in range(HD // 2):
        R[2 * i + 1, 2 * i] = -1.0
        R[2 * i, 2 * i + 1] = 1.0

    in_maps = []
    for c in range(NCORES):
        heads = range(HPC * c, HPC * (c + 1))
        cols = []
        for s in (0, 1, 2):  # q, k, v columns for this core's heads
            for h in heads:
                cols.append(np.arange(s * D + h * HD, s * D + (h + 1) * HD))
        w_qkv_c = np.ascontiguousarray(
            w_qkv[:, np.concatenate(cols)].astype(ml_dtypes.bfloat16)
        )
        # full-row w_proj, this core's output-column shard
        cshard = c % 2
        w_proj_c = np.ascontiguousarray(
            w_proj[:, cshard * COLW : (cshard + 1) * COLW].astype(ml_dtypes.bfloat16)
        )
        in_maps.append(
            {
                "xT": xT,
                "w_qkv": w_qkv_c,
                "w_proj": w_proj_c,
                "cos": cos_t,
                "sin": sin_t,
                "rot": R,
            }
        )
    return in_maps


_NC_CACHE = None


def kernel(x, w_qkv, w_proj):
    global _NC_CACHE
    if _NC_CACHE is None:
        _NC_CACHE = _build()
    nc = _NC_CACHE
    in_maps = _make_inputs(x, w_qkv, w_proj)
    res = run_bass_kernel_spmd(nc, in_maps, core_ids=list(range(NCORES)))
    out = np.empty((B, L, D), dtype=np.float32)
    for r in range(NCORES):
        o = res.results[r]["out"]  # [B*TOKG, COLW]
        g, cshard = r // 2, r % 2
        for b in range(B):
            out[
                b,
                g * TOKG : (g + 1) * TOKG,
                cshard * COLW : (cshard + 1) * COLW,
            ] = o[b * TOKG : (b + 1) * TOKG]
    return out.astype(np.float32)


# revision 21
# speedup vs baseline: 1.0078x; 1.0078x over previous
"""Distributed Trainium2 Bass kernel for nn_Attention_79766132621772.

Reference computation (all fp32):
    B, L, D, H, HD = 2, 2048, 2048, 16, 128
    qkv = (x @ w_qkv).reshape(B, L, 3, H, HD)
    q, k = rope(q), rope(k)                       # positions along L
    att = softmax(q @ k^T / sqrt(HD))             # per (b, h)
    out = (att @ v).reshape(B, L, D) @ w_proj

Sharding: tensor-parallel over heads for QKV+attention (16 heads / 8 cores =
2 heads per core), then an AllToAll redistributes the per-head attention
outputs into (token-group, output-column) shards for the projection: core r
projects token group r//2 (512 tokens per batch) against output columns
[1024*(r%2), 1024*(r%2+1)) with a resident half-w_proj. The AllToAll moves
only each core's 2-head oT slices (3.7 MB/core wire) instead of the 14.7
MB/core a ReduceScatter of full-D projection partials would need, and the
projection PE cost is unchanged. The host concatenates the 8 disjoint
(token, column) output shards.

Precision: bf16 operands everywhere upstream of the fp32 PSUM accumulators
(x, w_qkv, q/k/v, P, w_proj). Emulated end-to-end rel err 6e-3, and bf16
halves the x DMA stream and SBUF footprint vs fp32 while running at the same
1 cycle/row TensorE rate as fp32r.

Schedule: the PE is the bottleneck (roofline ~786k cycles/core). Stage 2 is
gated by ScalarE exp, so the emission order software-pipelines the engines:
the next batch's QKV matmuls and the previous batch's projection are
drip-fed as filler into the exp-gated stage-2 units, keeping the PE queue
fed while ScalarE chews exp. Nothing in the steady-state path touches
GpSimd (the collective occupies its in-order queue for the whole transfer,
so anything queued behind it would stall): row sums accumulate on DVE, and
the partition reduce uses a ones-MATRIX matmul whose output already
broadcasts the rowsum to all 128 partitions, so normalization is just a
DVE reciprocal + multiply. (Tried and reverted: fp8 DoubleRow — attention
averaging shrinks signal and noise equally, ~2.5-8%% output error; per-head
AllToAll splits — two extra collective launches cost more than the overlap
they buy; rot-matmul elimination via partition-offset swap copies — walrus
rejects the IR and the swap work gates whichever engine hosts it.)
"""

import os
import ml_dtypes
import numpy as np

import concourse.bass as bass
import concourse.tile as tile
from concourse import bacc, mybir
from concourse._compat import axon_active
from concourse.bass_utils import run_bass_kernel_spmd

B, L, D, H = 2, 2048, 2048, 16
HD = 128
NCORES = 8
HPC = H // NCORES          # heads per core = 2
T = B * L                  # total tokens = 4096
F32 = mybir.dt.float32
F32R = mybir.dt.float32r
BF16 = mybir.dt.bfloat16
SCALE = 1.0 / float(np.sqrt(HD))

_CHUNK = 512               # q/token chunk width (moving dim of matmuls)
_NKT = D // 128            # 16 contraction tiles for D=2048
_NCH = L // _CHUNK         # 4 chunks per batch
NTT = L // 128             # token tiles per batch = 16
TOKG = 512                 # tokens per group per batch (4 groups, 2 cores each)
COLW = D // 2              # output column shard width = 1024


def _build(reps=1, collective=True):
    # Native (non-axon) execution needs debug=True for the BassDebugger; the
    # axon/PJRT client path cannot host one and needs debug=False.
    nc = bacc.Bacc(
        "TRN2",
        target_bir_lowering=False,
        debug=not axon_active(),
        enable_asserts=False,
        num_devices=NCORES,
    )

    # ---- kernel I/O (per core) ----
    xT_d = nc.declare_dram_parameter("xT", [B, D, L], BF16, isOutput=False)
    wqkv_d = nc.declare_dram_parameter("w_qkv", [D, 6 * HD], BF16, isOutput=False)
    wproj_d = nc.declare_dram_parameter("w_proj", [D, COLW], BF16, isOutput=False)
    cos_d = nc.declare_dram_parameter("cos", [HD, L], BF16, isOutput=False)
    sin_d = nc.declare_dram_parameter("sin", [HD, L], BF16, isOutput=False)
    rot_d = nc.declare_dram_parameter("rot", [HD, HD], F32, isOutput=False)
    out_d = nc.declare_dram_parameter("out", [B * TOKG, COLW], F32, isOutput=True)

    with tile.TileContext(nc) as tc:
        _emit(nc, tc, xT_d, wqkv_d, wproj_d, cos_d, sin_d, rot_d, out_d, reps, collective)

    nc.compile()
    return nc


def _emit(nc, tc, xT_d, wqkv_d, wproj_d, cos_d, sin_d, rot_d, out_d, reps=1, collective=True):
    fdma = nc.sync.dma_start

    singles = tc.alloc_tile_pool(name="singles", bufs=1)
    # w_qkv in [128, kt, col] bf16 layout; cols: q_h0 q_h1 k_h0 k_h1 v_h0 v_h1
    w_sb = singles.tile([128, _NKT, 6 * HD], BF16)
    _wq_r = wqkv_d.ap().rearrange("(t p) c -> p t c", p=128)
    # startup-critical: only the first qk accumulation's own 128-column slice
    # of every k-tile; everything else is deferred until after the first x
    # chunk's DMAs are queued (the SP queue is in-order, so early non-critical
    # loads would delay the PE's first matmul)
    for _g in range(4):
        fdma(
            out=w_sb[:, 4 * _g : 4 * _g + 4, 0:128],
            in_=_wq_r[:, 4 * _g : 4 * _g + 4, 0:128],
        )
    wproj_sb = singles.tile([128, _NKT, COLW], BF16)
    cos_sb = singles.tile([HD, L], BF16)
    sin_sb = singles.tile([HD, L], BF16)
    rot_sb = singles.tile([HD, HD], F32R)
    ones_f32 = singles.tile([128, 128], F32)
    nc.vector.memset(ones_f32, 1.0)
    ones_sb = singles.tile([128, 128], F32R)
    nc.vector.tensor_copy(out=ones_sb, in_=ones_f32)

    def deferred_singles():
        fdma(out=rot_sb, in_=rot_d.ap().bitcast(F32R))
        for _cb in range(1, 6):
            fdma(
                out=w_sb[:, :, _cb * 128 : (_cb + 1) * 128],
                in_=_wq_r[:, :, _cb * 128 : (_cb + 1) * 128],
            )
        fdma(out=cos_sb, in_=cos_d.ap())
        fdma(out=sin_sb, in_=sin_d.ap())
        # w_proj is only needed at stage 3; emit its load last
        fdma(out=wproj_sb, in_=wproj_d.ap().rearrange("(t p) c -> p t c", p=128))

    # DRAM AllToAll buffers, one pair per batch: 8 blocks of [2*HD, TOKG];
    # block j carries this core's 2 heads x token group j//2 (destination j
    # projects those tokens against its column shard)
    dram = tc.alloc_tile_pool(name="dram", bufs=1, space="DRAM")
    a2a_in = dram.tile(
        [NCORES * HPC * HD, B * TOKG], BF16, tag="ain", name="a2a_in"
    )
    a2a_out = dram.tile(
        [NCORES * HPC * HD, B * TOKG], BF16, tag="aout", name="a2a_out"
    )

    # qT/kT/v double-buffered so batch b+1's stage 1 overlaps batch b's
    # attention
    per_b = tc.alloc_tile_pool(name="per_b", bufs=2)
    xp = tc.alloc_tile_pool(name="xp", bufs=int(os.environ.get("XG_BUFS", "19")))
    qs = tc.alloc_tile_pool(name="qs", bufs=int(os.environ.get("QS_BUFS", "2")))
    rp = tc.alloc_tile_pool(name="rp", bufs=int(os.environ.get("RP_BUFS", "2")))
    pp = tc.alloc_tile_pool(name="pp", bufs=int(os.environ.get("PP_BUFS", "4")))
    ap_ = tc.alloc_tile_pool(name="ap", bufs=2)
    op = tc.alloc_tile_pool(name="op", bufs=int(os.environ.get("OT_BUFS", "3")))
    at = tc.alloc_tile_pool(name="at", bufs=int(os.environ.get("AT_BUFS", "2")))
    ps_s = tc.alloc_tile_pool(name="ps_s", bufs=2, space="PSUM")
    ps_o = tc.alloc_tile_pool(name="ps_o", bufs=2, space="PSUM")
    ps_qkv = tc.alloc_tile_pool(name="ps_qkv", bufs=1, space="PSUM")
    ps_rot = tc.alloc_tile_pool(name="ps_rot", bufs=1, space="PSUM")
    ps_r = tc.alloc_tile_pool(name="ps_r", bufs=1, space="PSUM")

    state = {}  # (rep, b) -> dict of per-batch tiles

    def gen_s1(rep, b):
        """Stage 1 QKV+RoPE emitter; yields every ~2 PE matmuls so it can be
        drip-fed as filler into the exp-gated stage-2 units."""
        qT_sb = per_b.tile([128, HPC, L], BF16, tag="qT", name=f"qT_{rep}_{b}")
        kT_sb = per_b.tile([128, HPC, L], BF16, tag="kT", name=f"kT_{rep}_{b}")
        # v in [tok%128, tok_tile, head, HD] bf16 layout
        v_sb = per_b.tile([128, NTT, HPC, HD], BF16, tag="v", name=f"v_{rep}_{b}")
        state[(rep, b)] = dict(qT=qT_sb, kT=kT_sb, v=v_sb)

        xT_b = xT_d.ap()[b].rearrange("(t p) l -> p t l", p=128)  # [128,16,L]
        for ch in range(_NCH):
            c0 = ch * _CHUNK
            xg = []
            for g in range(_NKT):
                xgt = xp.tile([128, _CHUNK], BF16, tag=f"xg{b}", name=f"xg_{rep}_{b}_{ch}_{g}")
                fdma(out=xgt, in_=xT_b[:, g, c0 : c0 + _CHUNK])
                xg.append(xgt)

            # q/k in transposed [dim, token] layout, RoPE on eviction
            for ct in range(2 * HPC):
                dst = qT_sb if ct < HPC else kT_sb
                h = ct % HPC
                pq = ps_qkv.tile([128, _CHUNK], F32, tag=f"pqk{b}", name=f"pqk_{rep}_{b}_{ch}_{ct}")
                for kt in range(_NKT):
                    nc.tensor.matmul(
                        out=pq,
                        lhsT=w_sb[:, kt, ct * 128 : ct * 128 + 128],
                        rhs=xg[kt],
                        start=(kt == 0),
                        stop=(kt == _NKT - 1),
                    )
                    if kt % 2 == 1:
                        yield
                # evict, then rot = P_rot @ q via constant matmul
                qsb = qs.tile([128, _CHUNK], F32R, tag="qsb", name=f"qsb_{rep}_{b}_{ch}_{ct}")
                nc.scalar.copy(out=qsb, in_=pq)
                prot = ps_rot.tile(
                    [128, _CHUNK], F32, tag="prot", name=f"prot_{rep}_{b}_{ch}_{ct}"
                )
                nc.tensor.matmul(out=prot, lhsT=rot_sb, rhs=qsb, start=True, stop=True)
                # q' = q*cos + rot*sin (single bf16 rounding at the end)
                cosc = cos_sb[:, c0 : c0 + _CHUNK]
                sinc = sin_sb[:, c0 : c0 + _CHUNK]
                dstc = dst[:, h, c0 : c0 + _CHUNK]
                t1 = rp.tile([128, _CHUNK], F32, tag="rt", name=f"rt_{rep}_{b}_{ch}_{ct}")
                t2 = rp.tile([128, _CHUNK], F32, tag="rt2", name=f"rt2_{rep}_{b}_{ch}_{ct}")
                nc.vector.tensor_mul(out=t1, in0=prot, in1=sinc)
                nc.vector.tensor_mul(out=t2, in0=qsb.bitcast(F32), in1=cosc)
                nc.vector.tensor_add(out=dstc, in0=t2, in1=t1)
                yield

            # v in [token, col] bf16 layout
            for tt in range(_CHUNK // 128):
                pv = ps_qkv.tile([128, HPC * HD], F32, tag=f"pqk{b}", name=f"pv_{rep}_{b}_{ch}_{tt}")
                for kt in range(_NKT):
                    nc.tensor.matmul(
                        out=pv,
                        lhsT=xg[kt][:, tt * 128 : tt * 128 + 128],
                        rhs=w_sb[:, kt, 2 * HPC * 128 :],
                        start=(kt == 0),
                        stop=(kt == _NKT - 1),
                    )
                    if kt % 4 == 3:
                        yield
                gt = ch * (_CHUNK // 128) + tt
                nc.scalar.copy(
                    out=v_sb[:, gt, :, :].rearrange("p h d -> p (h d)"), in_=pv
                )
                yield

    fil = []  # deque of filler generators

    def take(n):
        while n > 0 and fil:
            try:
                next(fil[0])
                n -= 1
            except StopIteration:
                fil.pop(0)

    def emit_s2_unit(rep, b, h, qc):
        st = state[(rep, b)]
        q0 = qc * _CHUNK
        qT_c = st["qT"][:, h, q0 : q0 + _CHUNK]
        po = ps_o.tile([128, _CHUNK], F32, tag="po", name=f"po_{rep}_{b}_{h}_{qc}")
        acc = ap_.tile([128, _CHUNK], F32R, tag="acc", name=f"acc_{rep}_{b}_{h}_{qc}")
        for kt in range(_NKT):
            psS = ps_s.tile([128, _CHUNK], F32, tag="ps", name=f"ps_{rep}_{b}_{h}_{qc}_{kt}")
            nc.tensor.matmul(
                out=psS,
                lhsT=st["kT"][:, h, kt * 128 : kt * 128 + 128],
                rhs=qT_c,
                start=True,
                stop=True,
            )
            pt = pp.tile([128, _CHUNK], BF16, tag="pt", name=f"pt_{rep}_{b}_{h}_{qc}_{kt}")
            nc.scalar.activation(
                out=pt, in_=psS, func=mybir.ActivationFunctionType.Exp,
                scale=SCALE,
            )
            nc.tensor.matmul(
                out=po,
                lhsT=st["v"][:, kt, h, :],
                rhs=pt,
                start=(kt == 0),
                stop=(kt == _NKT - 1),
            )
            # row-sum accumulation on DVE (NOT GpSimd: the collective
            # occupies the in-order GpSimd queue for its whole duration)
            if kt == 0:
                nc.vector.tensor_copy(out=acc, in_=pt)
            else:
                nc.vector.tensor_add(out=acc, in0=acc.bitcast(F32), in1=pt)
            take(1)
        # partition-reduce the column sums with a ones-MATRIX matmul: every
        # output partition gets the same rowsum, so no GpSimd broadcast is
        # needed (GpSimd stays free for the AllToAll collectives)
        pr = ps_r.tile([128, _CHUNK], F32, tag="pr", name=f"pr_{rep}_{b}_{h}_{qc}")
        nc.tensor.matmul(out=pr, lhsT=ones_sb, rhs=acc, start=True, stop=True)
        rbc = rp.tile([128, _CHUNK], F32, tag="rbc", name=f"rbc_{rep}_{b}_{h}_{qc}")
        nc.vector.reciprocal(out=rbc, in_=pr)
        ot = op.tile([128, _CHUNK], BF16, tag="ot", name=f"ot_{rep}_{b}_{h}_{qc}")
        nc.vector.tensor_mul(out=ot, in0=po, in1=rbc)
        # ship this unit's oT slice to both column-shard owners of token
        # group qc: AllToAll block rows [256*j + 128*h, +128) for j = 2qc,
        # 2qc+1
        for j in (2 * qc, 2 * qc + 1):
            r0 = (HPC * HD) * j + HD * h
            fdma(
                out=a2a_in[r0 : r0 + HD, b * TOKG : (b + 1) * TOKG], in_=ot
            )

    def emit_a2a(rep):
        if collective:
            nc.gpsimd.collective_compute(
                "AllToAll",
                mybir.AluOpType.bypass,
                replica_groups=[list(range(NCORES))],
                ins=[a2a_in.opt()],
                outs=[a2a_out.opt()],
            )

    def gen_proj(rep, b):
        """Project this core's token group of batch b (512 tokens x COLW
        columns, full-D contraction). Each quantum is atomic per token tile
        (no pool tile stays live across a yield, so interleaved emission
        from other generators can never create a forward slot-wait)."""
        src = a2a_out if collective else a2a_in
        src_r = src.rearrange("(t p) c -> p t c", p=128)  # [128,16,B*TOKG]
        for tt in range(TOKG // 128):
            a2a_t = at.tile([128, _NKT, 128], BF16, tag="at", name=f"at_{rep}_{b}_{tt}")
            c0 = b * TOKG + tt * 128
            fdma(out=a2a_t, in_=src_r[:, :, c0 : c0 + 128])
            for nch in range(COLW // _CHUNK):
                pout = ps_s.tile([128, _CHUNK], F32, tag="ps", name=f"pout_{rep}_{b}_{tt}_{nch}")
                for kt in range(_NKT):
                    nc.tensor.matmul(
                        out=pout,
                        lhsT=a2a_t[:, kt, :],
                        rhs=wproj_sb[:, kt, nch * _CHUNK : (nch + 1) * _CHUNK],
                        start=(kt == 0),
                        stop=(kt == _NKT - 1),
                    )
                fout = qs.tile([128, _CHUNK], F32, tag="fout", name=f"fo_{rep}_{b}_{tt}_{nch}")
                # alternate eviction between ScalarE and VectorE
                if nch % 2 == 0:
                    nc.scalar.copy(out=fout, in_=pout)
                else:
                    nc.vector.tensor_copy(out=fout, in_=pout)
                fdma(
                    out=out_d.ap()[
                        b * TOKG + tt * 128 : b * TOKG + (tt + 1) * 128,
                        nch * _CHUNK : (nch + 1) * _CHUNK,
                    ],
                    in_=fout,
                )
            yield

    gens1 = {}

    def s1g(rep, b):
        if (rep, b) not in gens1:
            gens1[(rep, b)] = gen_s1(rep, b)
        return gens1[(rep, b)]

    for rep in range(reps):
        if rep == 0:
            next(s1g(0, 0))      # queue the first x chunk's DMAs first
            deferred_singles()   # then the non-critical parameter loads
        for _ in s1g(rep, 0):    # finish stage 1 of b0, draining queued proj
            take(1)
        for b in range(B):
            # queue the next batch's stage 1 as filler for this batch's
            # exp-gated attention units
            nrep, nb = (rep, 1) if b == 0 else (rep + 1, 0)
            if nrep < reps:
                fil.append(s1g(nrep, nb))
            for qc in range(_NCH):
                for h in range(HPC):
                    emit_s2_unit(rep, b, h, qc)
            if b == 0:
                for _ in s1g(rep, 1):
                    take(1)
            else:
                emit_a2a(rep)
                # prioritize the projections over the next stage 1
                fil.insert(0, gen_proj(rep, 0))
                fil.insert(1, gen_proj(rep, 1))
        # drain any remaining filler at the rep boundary? No — leave it for
        # the next rep's stage-2 gaps; force-drain only at the very end.
    while fil:
        take(100)

    for p in (ps_r, ps_rot, ps_qkv, ps_o, ps_s, at, op, ap_, pp, rp, qs, xp, per_b, dram, singles):
        p.release()


def _make_inputs(x, w_qkv, w_proj):
    x = np.asarray(x, dtype=np.float32)
    w_qkv = np.asarray(w_qkv, dtype=np.float32)
    w_proj = np.asarray(w_proj, dtype=np.float32)
    xT = np.ascontiguousarray(x.transpose(0, 2, 1)).astype(ml_dtypes.bfloat16)

    freqs = (1.0 / (10000.0 ** (np.arange(0, HD, 2, dtype=np.float32) / HD))).astype(
        np.float32
    )
    f = np.outer(np.arange(L, dtype=np.float32), freqs).astype(np.float32)  # [L, 64]
    cos_t = np.ascontiguousarray(np.repeat(np.cos(f), 2, axis=1).T.astype(ml_dtypes.bfloat16))
    sin_t = np.ascontiguousarray(np.repeat(np.sin(f), 2, axis=1).T.astype(ml_dtypes.bfloat16))

    # rot param R = P_rot^T, where rot(q) = P_rot @ q swaps pairs:
    # rot[2i] = -q[2i+1], rot[2i+1] = q[2i]
    R = np.zeros((HD, HD), dtype=np.float32)
    for i in range(HD // 2):
        R[2 * i + 1, 2 * i] = -1.0
        R[2 * i, 2 * i + 1] = 1.0

    in_maps = []
    for c in range(NCORES):
        heads = range(HPC * c, HPC * (c + 1))
        cols = []
        for s in (0, 1, 2):  # q, k, v columns for this core's heads
            for h in heads:
                cols.append(np.arange(s * D + h * HD, s * D + (h + 1) * HD))
        w_qkv_c = np.ascontiguousarray(
            w_qkv[:, np.concatenate(cols)].astype(ml_dtypes.bfloat16)
        )
        # full-row w_proj, this core's output-column shard
        cshard = c % 2
        w_proj_c = np.ascontiguousarray(
            w_proj[:, cshard * COLW : (cshard + 1) * COLW].astype(ml_dtypes.bfloat16)
        )
        in_maps.append(
            {
                "xT": xT,
                "w_qkv": w_qkv_c,
                "w_proj": w_proj_c,
                "cos": cos_t,
                "sin": sin_t,
                "rot": R,
            }
        )
    return in_maps


_NC_CACHE = None


def kernel(x, w_qkv, w_proj):
    global _NC_CACHE
    if _NC_CACHE is None:
        _NC_CACHE = _build()
    nc = _NC_CACHE
    in_maps = _make_inputs(x, w_qkv, w_proj)
    res = run_bass_kernel_spmd(nc, in_maps, core_ids=list(range(NCORES)))
    out = np.empty((B, L, D), dtype=np.float32)
    for r in range(NCORES):
        o = res.results[r]["out"]  # [B*TOKG, COLW]
        g, cshard = r // 2, r % 2
        for b in range(B):
            out[
                b,
                g * TOKG : (g + 1) * TOKG,
                cshard * COLW : (cshard + 1) * COLW,
            ] = o[b * TOKG : (b + 1) * TOKG]
    return out.astype(np.float32)
